# revision 4
# baseline (speedup 1.0000x reference)
"""MoE (16 routed experts, top-2, + shared expert) on 8 Trainium2 cores — v2.

Expert-parallel with host-side routing (gate + gather/scatter on host):
  - Core c owns 2 routed experts (host assigns expert pairs to balance the
    token load); tokens for slot A occupy x columns [0, CA), slot B
    [CA, CA+CB), where CA/CB are the max padded counts across cores.
  - Routed experts run in scaled e4m3 fp8 with DoubleRow matmuls
    (~1.7x PE rate, half the weight bytes).  Their outputs are multiplied
    by the top-2 softmax weights (~0.1-0.3), so fp8 noise is attenuated
    ~5x in the final output; measured rel err ~1.64e-2 vs the 2e-2 gate.
  - The shared expert (full-scale contribution) stays in f16.
  - All outputs return as f16; host applies combine weights and
    scatter-adds in f32.
  - DMAs are consolidated (whole-tensor weight loads, one staged output
    DMA per block) because every dma_start serializes ~0.6us on the
    shared HWDGE dispatcher.

Layouts are transposed (tokens along the matmul free dim):
    zT = W2^T @ (u * silu(g)),  [u;g]^T = W1^T @ xT
so no on-chip transposes are needed.  Weights arrive pre-packed per-core
from the host in the exact SBUF tile layout (contiguous per-partition runs).
"""

import sys

for _p in ("/opt/trn_rl_repo", "/root/.axon_site/_ro/trn_rl_repo"):
    if _p not in sys.path:
        sys.path.insert(0, _p)

import contextlib
import os

import numpy as np
import ml_dtypes

import concourse.bass as bass  # noqa: F401
import concourse.tile as tile
from concourse import bacc, mybir
from concourse.bass_utils import run_bass_kernel_spmd

try:
    from antenv import axon_hooks as _axon_hooks  # noqa: F401
except ImportError:
    os.environ.setdefault("BASS_NEVER_TRACE", "1")

B, S, D = 2, 1024, 1024
H = 512            # routed expert hidden
HS = 1024          # shared expert hidden
E = 16
ROUTE_SCALE = 1.0
T = B * S
N_CORES = 8
EPC = E // N_CORES
TDP = T // N_CORES          # shared-expert tokens per core
P = 128
KD = D // P                 # fc1 contraction chunks (8)
MD = D // P                 # fc2 output chunks (8)
NH = H // P                 # routed hidden chunks (4)
NHS = HS // P               # shared hidden chunks (8)
PAD = 16                    # token-count granularity (DoubleRow step%16)
CBLK = 512                  # max psum free dim (one 2KB bank of f32)

SW1 = 16.0                  # fp8 pre-scales (keep |h| < 240 for ieee e4m3)
SXR = 1.0
SW2 = 16.0

F32 = mybir.dt.float32
F16 = mybir.dt.float16
E4 = mybir.dt.float8e4
NP_E4 = ml_dtypes.float8_e4m3
ACT = mybir.ActivationFunctionType

LAST_RESULTS = None
_NC_CACHE = {}


def _blocks(cap):
    """Split cap columns into blocks of <= CBLK, sizes multiple of PAD."""
    nb = -(-cap // CBLK)
    base = -(-cap // (nb * PAD)) * PAD
    out = []
    off = 0
    while off < cap:
        n = min(base, cap - off)
        out.append((off, n))
        off += n
    return out


def _build_nc(CA, CB, reps=1, static_loop=False, only=None):
    # only='dma' -> input DMAs only; only='compute' -> inputs hoisted out of loop
    nc = bacc.Bacc(None, target_bir_lowering=False)
    C2 = CA + CB

    xr = nc.declare_dram_parameter("xr", [P, KD * C2], E4, isOutput=False)
    xs = nc.declare_dram_parameter("xs", [P, KD * TDP], F16, isOutput=False)
    W1N = NH * 2 * (KD // 2) * 2 * P
    W2N = MD * (NH // 2) * 2 * P
    WS1N = NHS * 2 * KD * P
    WS2N = (MD // 2) * 2 * NHS * P
    w1r = [nc.declare_dram_parameter(f"w1e{i}", [P, W1N], E4, isOutput=False)
           for i in range(EPC)]
    w2r = [nc.declare_dram_parameter(f"w2e{i}", [P, W2N], E4, isOutput=False)
           for i in range(EPC)]
    ws1 = nc.declare_dram_parameter("ws1", [P, WS1N], F16, isOutput=False)
    ws2 = nc.declare_dram_parameter("ws2", [P, WS2N], F16, isOutput=False)
    zr = [nc.declare_dram_parameter(f"zr{i}", [P, MD * (CA, CB)[i]], F16, isOutput=True)
          for i in range(EPC)]
    zs = nc.declare_dram_parameter("zs", [P, MD * TDP], F16, isOutput=True)

    slot_cols = [(0, CA), (CA, CB)]

    def v_w1(ap):
        return ap.rearrange("p (h s k j m) -> p h s k j m", h=NH, s=2, k=KD // 2, j=2)

    def v_w2(ap):
        return ap.rearrange("p (d k j m) -> p d k j m", d=MD, k=NH // 2, j=2)

    def v_ws1(ap):
        return ap.rearrange("p (h s k m) -> p h s k m", h=NHS, s=2, k=KD)

    def v_ws2(ap):
        return ap.rearrange("p (d s k m) -> p d s k m", d=MD // 2, s=2, k=NHS)

    with tile.TileContext(nc) as tc:
        with (
            tc.tile_pool(name="xpool", bufs=2) as xpool,
            tc.tile_pool(name="wrpool", bufs=2) as wrpool,
            tc.tile_pool(name="ws1pool", bufs=2) as ws1pool,
            tc.tile_pool(name="ws2pool", bufs=1) as ws2pool,
            tc.tile_pool(name="hpool", bufs=2) as hpool,
            tc.tile_pool(name="spool", bufs=4) as spool,
            tc.tile_pool(name="opool", bufs=2) as opool,
            tc.tile_pool(name="psu", bufs=2, space="PSUM") as psu,
            tc.tile_pool(name="psg", bufs=2, space="PSUM") as psg,
            tc.tile_pool(name="psz", bufs=3, space="PSUM") as psz,
        ):
            pre = {}
            if only == "compute":
                t = xpool.tile([P, KD * C2], E4, tag="xr", name="pre_xr")
                nc.sync.dma_start(t[:], xr.ap())
                pre["xr"] = t[:].rearrange("p (k c) -> p k c", k=KD)
                t = xpool.tile([P, KD * TDP], F16, tag="xs", name="pre_xs")
                nc.sync.dma_start(t[:], xs.ap())
                pre["xs"] = t[:].rearrange("p (k c) -> p k c", k=KD)
                for i in range(EPC):
                    t = wrpool.tile([P, W1N], E4, tag=f"w1_{i}", name=f"pre_w1_{i}")
                    nc.sync.dma_start(t[:], w1r[i].ap())
                    pre[f"w1_{i}"] = v_w1(t[:])
                    t = wrpool.tile([P, W2N], E4, tag=f"w2_{i}", name=f"pre_w2_{i}")
                    nc.sync.dma_start(t[:], w2r[i].ap())
                    pre[f"w2_{i}"] = v_w2(t[:])
                t = ws1pool.tile([P, WS1N], F16, tag="ws1", name="pre_ws1")
                nc.sync.dma_start(t[:], ws1.ap())
                pre["ws1"] = v_ws1(t[:])
                t = ws2pool.tile([P, WS2N], F16, tag="ws2", name="pre_ws2")
                nc.sync.dma_start(t[:], ws2.ap())
                pre["ws2"] = v_ws2(t[:])
            if reps > 1 and not static_loop:
                # unroll x2 inside the hw loop: consecutive iterations then
                # alternate pool buffers (bufs=2), enabling cross-iteration
                # DMA/compute overlap that a single-emission dynamic loop
                # cannot express.
                assert reps % 2 == 0
                n_emit = 2
                loop_cm = tc.For_i(0, reps // 2, 1)
            else:
                n_emit = reps if (static_loop and reps > 1) else 1
                loop_cm = contextlib.nullcontext()
            with loop_cm:
              for _rep in range(n_emit):
                if only == "compute":
                    xr_t, xs_t = pre["xr"], pre["xs"]
                else:
                    t = xpool.tile([P, KD * C2], E4, tag="xr", name="xr_t")
                    nc.sync.dma_start(t[:], xr.ap())
                    xr_t = t[:].rearrange("p (k c) -> p k c", k=KD)
                    t = xpool.tile([P, KD * TDP], F16, tag="xs", name="xs_t")
                    nc.sync.dma_start(t[:], xs.ap())
                    xs_t = t[:].rearrange("p (k c) -> p k c", k=KD)

                # ---- weight DMAs up front, split across SP and Act queues ----
                # SP queue: w1e0, w2e0, ws1 (+xr above); Act queue: w1e1, w2e1,
                # ws2 (+xs above, + output DMAs)
                if only == "compute":
                    w1ts = [pre["w1_0"], pre["w1_1"]]
                    w2ts = [pre["w2_0"], pre["w2_1"]]
                    ws1t, ws2t = pre["ws1"], pre["ws2"]
                else:
                    w1ts, w2ts = [], []
                    for slot in range(EPC):
                        q = nc.sync if slot == 0 else nc.scalar
                        t = wrpool.tile([P, W1N], E4, tag=f"w1_{slot}",
                                        name=f"w1t{slot}")
                        q.dma_start(t[:], w1r[slot].ap())
                        w1ts.append(v_w1(t[:]))
                        t = wrpool.tile([P, W2N], E4, tag=f"w2_{slot}",
                                        name=f"w2t{slot}")
                        q.dma_start(t[:], w2r[slot].ap())
                        w2ts.append(v_w2(t[:]))
                    t = ws1pool.tile([P, WS1N], F16, tag="ws1", name="ws1t")
                    nc.sync.dma_start(t[:], ws1.ap())
                    ws1t = v_ws1(t[:])
                    t = ws2pool.tile([P, WS2N], F16, tag="ws2", name="ws2t")
                    nc.scalar.dma_start(t[:], ws2.ap())
                    ws2t = v_ws2(t[:])
                if only == "dma":
                    os_t = opool.tile([P, MD, TDP], F16, tag="os")
                    nc.vector.memset(os_t[:], 0.0)
                    nc.scalar.dma_start(zs.ap(), os_flat[:])
                    continue

                def fc1_routed(slot):
                    coff, cap = slot_cols[slot]
                    h_t = hpool.tile([P, NH, cap], E4, tag=f"hr{slot}",
                                     name=f"h{slot}")
                    for hc in range(NH):
                        for boff, bn in _blocks(cap):
                            ps_u = psu.tile([P, bn], F32, tag="psu", name="ps_u")
                            ps_g = psg.tile([P, bn], F32, tag="psg", name="ps_g")
                            for ps, half in ((ps_u, 0), (ps_g, 1)):
                                for kp in range(KD // 2):
                                    nc.tensor.matmul(
                                        ps[:], w1ts[slot][:, hc, half, kp],
                                        xr_t[:, 2 * kp:2 * kp + 2,
                                             coff + boff:coff + boff + bn],
                                        start=(kp == 0), stop=(kp == KD // 2 - 1),
                                        perf_mode=mybir.MatmulPerfMode.DoubleRow)
                            sil = spool.tile([P, bn], F32, tag="sil", name="sil")
                            nc.scalar.activation(sil[:], ps_g[:], ACT.Silu,
                                                 scale=1.0 / (SW1 * SXR))
                            nc.vector.tensor_mul(
                                h_t[:, hc, boff:boff + bn], ps_u[:], sil[:])
                    return h_t

                def fc2_routed(slot, h_t):
                    coff, cap = slot_cols[slot]
                    o_flat = opool.tile([P, MD * cap], F16, tag=f"or{slot}",
                                        name=f"o{slot}")
                    o_t = o_flat[:].rearrange("p (d c) -> p d c", d=MD)
                    for boff, bn in _blocks(cap):
                        for dp in range(MD):
                            ps_z = psz.tile([P, bn], F32, tag="psz", name="ps_z")
                            for kp in range(NH // 2):
                                nc.tensor.matmul(
                                    ps_z[:], w2ts[slot][:, dp, kp],
                                    h_t[:, 2 * kp:2 * kp + 2, boff:boff + bn],
                                    start=(kp == 0), stop=(kp == NH // 2 - 1),
                                    perf_mode=mybir.MatmulPerfMode.DoubleRow)
                            if dp % 2 == 0:
                                nc.scalar.activation(o_t[:, dp, boff:boff + bn],
                                                     ps_z[:], ACT.Copy,
                                                     scale=1.0 / (SW1 * SXR * SW2))
                            else:
                                nc.vector.tensor_scalar_mul(
                                    o_t[:, dp, boff:boff + bn], ps_z[:],
                                    1.0 / (SW1 * SXR * SW2))
                    nc.scalar.dma_start(zr[slot].ap(), o_flat[:])

                def fc1_shared():
                    hs_t = hpool.tile([P, NHS, TDP], F16, tag="hs", name="hs")
                    for hc in range(NHS):
                        ps_u = psu.tile([P, TDP], F32, tag="psu", name="ps_u")
                        ps_g = psg.tile([P, TDP], F32, tag="psg", name="ps_g")
                        for ps, half in ((ps_u, 0), (ps_g, 1)):
                            for k in range(KD):
                                nc.tensor.matmul(ps[:], ws1t[:, hc, half, k],
                                                 xs_t[:, k],
                                                 start=(k == 0), stop=(k == KD - 1))
                        sil = spool.tile([P, TDP], F32, tag="sil", name="sil")
                        nc.scalar.activation(sil[:], ps_g[:], ACT.Silu)
                        nc.vector.tensor_mul(hs_t[:, hc], ps_u[:], sil[:])
                    return hs_t

                def fc2_shared(hs_t):
                    os_flat = opool.tile([P, MD * TDP], F16, tag="os", name="os")
                    os_t = os_flat[:].rearrange("p (d c) -> p d c", d=MD)
                    for dpp in range(MD // 2):
                        for s2 in range(2):
                            dp = 2 * dpp + s2
                            ps_z = psz.tile([P, TDP], F32, tag="psz", name="ps_z")
                            for k in range(NHS):
                                nc.tensor.matmul(ps_z[:], ws2t[:, dpp, s2, k],
                                                 hs_t[:, k],
                                                 start=(k == 0), stop=(k == NHS - 1))
                            nc.vector.tensor_copy(os_t[:, dp], ps_z[:])
                    nc.scalar.dma_start(zs.ap(), os_flat[:])

                # ordering: cover each fc1->fc2 junction with independent work
                h0 = fc1_routed(0)
                h1 = fc1_routed(1)
                fc2_routed(0, h0)
                hs_t = fc1_shared()
                fc2_routed(1, h1)
                fc2_shared(hs_t)
    nc.finalize()
    return nc


def _route(xf, Wg):
    logits = xf @ Wg.T
    m = logits.max(axis=-1, keepdims=True)
    p = np.exp(logits - m)
    scores = p / p.sum(axis=-1, keepdims=True)
    i1 = scores.argmax(axis=-1)
    rows = np.arange(T)
    s1 = scores[rows, i1]
    masked = scores.copy()
    masked[rows, i1] = -np.inf
    i2 = masked.argmax(axis=-1)
    s2 = scores[rows, i2]
    return i1, s1 * ROUTE_SCALE, i2, s2 * ROUTE_SCALE


def _pack_w1_r(W1e):
    """[D, 2H] -> [P, NH, 2, KD/2, 2, P] e4m3 scaled (pair-interleaved k)."""
    A = (W1e * SW1).reshape(KD // 2, 2, P, 2, NH, P)   # [kp, j, ki, half, hc, m]
    return np.ascontiguousarray(
        A.transpose(2, 4, 3, 0, 1, 5).reshape(P, -1)
    ).astype(NP_E4)


def _pack_w2_r(W2e):
    """[H, D] -> [P, MD, NH/2, 2, P] e4m3 scaled."""
    A = (W2e * SW2).reshape(NH // 2, 2, P, MD, P)      # [kp, j, ki, dp, m]
    return np.ascontiguousarray(
        A.transpose(2, 3, 0, 1, 4).reshape(P, -1)
    ).astype(NP_E4)


def _pack_ws1(Ws1):
    """[D, 2HS] -> [P, NHS, 2, KD, P] f16."""
    A = Ws1.reshape(KD, P, 2, NHS, P)                  # [ko, ki, half, hc, m]
    return np.ascontiguousarray(
        A.transpose(1, 3, 2, 0, 4).reshape(P, -1)).astype(np.float16)


def _pack_ws2(Ws2):
    """[HS, D] -> [P, MD/2, 2, NHS, P] f16."""
    A = Ws2.reshape(NHS, P, MD // 2, 2, P)             # [ko, ki, dpp, s2, m]
    return np.ascontiguousarray(
        A.transpose(1, 2, 3, 0, 4).reshape(P, -1)).astype(np.float16)


def _pack_x(cols_f32, C, dtype, scale):
    """[D, n] -> [P, KD, C] (zero-padded to C columns)."""
    n = cols_f32.shape[1]
    out = np.zeros((P, KD, C), dtype=dtype)
    v = (cols_f32 * scale).reshape(KD, P, n).transpose(1, 0, 2)
    out[:, :, :n] = v.astype(dtype)
    return out


def prepare(x, Wg, W1, W2, Ws1, Ws2):
    """Host routing, balancing, packing. Returns (in_maps, meta)."""
    x = np.asarray(x, dtype=np.float32)
    Wg = np.asarray(Wg, dtype=np.float32)
    W1 = np.asarray(W1, dtype=np.float32)
    W2 = np.asarray(W2, dtype=np.float32)
    Ws1 = np.asarray(Ws1, dtype=np.float32)
    Ws2 = np.asarray(Ws2, dtype=np.float32)

    xf = np.ascontiguousarray(x.reshape(T, D))
    i1, s1, i2, s2 = _route(xf, Wg)

    toks, wts = [], []
    for e in range(E):
        sel = np.where((i1 == e) | (i2 == e))[0]
        toks.append(sel)
        wts.append(np.where(i1[sel] == e, s1[sel], s2[sel]).astype(np.float32))

    counts = np.array([len(t) for t in toks])
    order = np.argsort(-counts)                     # big..small
    slotA = [int(order[c]) for c in range(N_CORES)]           # biggest 8
    slotB = [int(order[2 * N_CORES - 1 - c]) for c in range(N_CORES)]
    npad = [-(-c // PAD) * PAD for c in counts]
    CA = max(PAD, max(npad[e] for e in slotA))
    CB = max(PAD, max(npad[e] for e in slotB))

    ws1p = _pack_ws1(Ws1)
    ws2p = _pack_ws2(Ws2)
    in_maps = []
    for c in range(N_CORES):
        eA, eB = slotA[c], slotB[c]
        xcat = np.zeros((P, KD, CA + CB), dtype=NP_E4)
        xcat[:, :, :CA] = _pack_x(xf[toks[eA]].T, CA, NP_E4, SXR)
        xcat[:, :, CA:] = _pack_x(xf[toks[eB]].T, CB, NP_E4, SXR)
        im = {
            "xr": xcat.reshape(P, KD * (CA + CB)),
            "xs": _pack_x(xf[c * TDP:(c + 1) * TDP].T, TDP, np.float16,
                          1.0).reshape(P, KD * TDP),
            "ws1": ws1p, "ws2": ws2p,
            "w1e0": _pack_w1_r(W1[eA]), "w2e0": _pack_w2_r(W2[eA]),
            "w1e1": _pack_w1_r(W1[eB]), "w2e1": _pack_w2_r(W2[eB]),
        }
        in_maps.append(im)
    meta = dict(CA=CA, CB=CB, slotA=slotA, slotB=slotB, toks=toks, wts=wts)
    return in_maps, meta


def kernel(x, Wg, W1, W2, Ws1, Ws2):
    global LAST_RESULTS
    in_maps, meta = prepare(x, Wg, W1, W2, Ws1, Ws2)
    CA, CB = meta["CA"], meta["CB"]

    key = (CA, CB)
    if key not in _NC_CACHE:
        _NC_CACHE[key] = _build_nc(CA, CB)
    nc = _NC_CACHE[key]

    try:
        LAST_RESULTS = run_bass_kernel_spmd(nc, in_maps, list(range(N_CORES)))
    except Exception:
        LAST_RESULTS = run_bass_kernel_spmd(nc, in_maps, list(range(N_CORES)))
    res = LAST_RESULTS.results

    toks, wts = meta["toks"], meta["wts"]
    out = np.zeros((T, D), dtype=np.float32)
    for c in range(N_CORES):
        for slot, e in ((0, meta["slotA"][c]), (1, meta["slotB"][c])):
            n = len(toks[e])
            cap = (CA, CB)[slot]
            zrf = np.asarray(res[c][f"zr{slot}"], dtype=np.float32).reshape(P, MD, cap)
            zt = zrf[:, :, :n].transpose(1, 0, 2).reshape(D, n)       # [D, n]
            out[toks[e]] += wts[e][:, None] * zt.T
        zsf = np.asarray(res[c]["zs"], dtype=np.float32).reshape(P, MD, TDP)
        out[c * TDP:(c + 1) * TDP] += zsf.transpose(1, 0, 2).reshape(D, TDP).T
    return out.reshape(B, S, D)
